# revision 14
# baseline (speedup 1.0000x reference)
"""Trainium2 Bass kernel for nn_Attention_47682726920277.

Causal multi-head attention with RoPE:
  q/k/v = x @ w{q,k,v}.T ; RoPE(q, k) ; att = softmax(mask(q k^T / 8)) ; out = (att v) @ wo.T
Shapes: x [2, 2048, 1024], 16 heads of dim 64, fp32.

Sharding (8 cores): data-parallel over batch (2) x tensor-parallel over heads (4 per
core). Each core computes its 4 heads' attention and a partial out via its wo row
block; the final all-reduce is the host-side sum of the 4 partials per batch.

Per-core pipeline (single fused program):
  for m in 0..3 (query chunks of 512):
    project+RoPE q,k chunk m (both head-pairs); project v blocks 4m..4m+3
    attention(hp0, qc=m); normalize(hp0); wo chunk m-1; attention(hp1, qc=m);
    normalize(hp1)
  wo chunk 3
Scores come out transposed (S^T [k, q]) so the softmax denominator is the 65th
row of the PV accumulator (V augmented with a ones column). exp runs on the
scalar engine straight out of PSUM with the 1/8 scale fused and a strided access
pattern that skips causally-masked columns. Normalization is
reciprocal_approx_fast (DVE) + gpsimd partition-broadcast + one DVE multiply
reading the PV accumulator directly from PSUM. Score half-pairs row-pack onto
the PE array (tile_position auto via base partitions 0/64).
"""
import sys
import types
import numpy as np

B = 2
T = 2048
D = 1024
H = 16
HD = 64
NCORES = 8
GROUPS = NCORES // B          # head-groups per batch
HPC = H // GROUPS             # heads per core = 4
CH = HPC * HD                 # channels per core = 256
NQ = 512                      # PSUM bank width (fp32)
P = 128

_prog_cache = {}


def _install_ntff_shim():
    """The agent image's antenv lacks axon_hooks; inject it so trace=True works."""
    try:
        import antenv.axon_hooks  # noqa: F401
        return
    except ImportError:
        pass
    try:
        import trn_agent_boot.trn_boot as tb
        hook = tb._ntff_profile_via_ctypes('/opt/axon/libaxon_pjrt.so')
        if hook is None:
            return
        mod = types.ModuleType('antenv.axon_hooks')
        mod.get_axon_ntff_profile_hook = lambda: hook
        mod.set_axon_ntff_profile_hook = lambda h: None
        sys.modules['antenv.axon_hooks'] = mod
        import antenv
        antenv.axon_hooks = mod
    except Exception:
        pass


def _build_program(causal: bool):
    import concourse.bass as bass  # noqa: F401
    from concourse import bacc
    import concourse.tile as tile
    from concourse import mybir

    F32 = mybir.dt.float32
    F16 = mybir.dt.float16
    AF = mybir.ActivationFunctionType
    MUL = mybir.AluOpType.mult
    ADD = mybir.AluOpType.add

    NT = T // NQ          # proj/attention q-chunks (4)
    NKB = T // P          # k-blocks (16)
    DB = D // P           # d-blocks (8)
    CB = CH // P          # channel blocks = head-pair blocks (2)

    nc = bacc.Bacc("TRN2", target_bir_lowering=False, debug=False)

    xT = nc.dram_tensor("xT", [D, T], F16, kind="ExternalInput").ap()
    wqT = nc.dram_tensor("wqT", [D, CH], F16, kind="ExternalInput").ap()
    wkT = nc.dram_tensor("wkT", [D, CH], F16, kind="ExternalInput").ap()
    wvT = nc.dram_tensor("wvT", [D, CH], F16, kind="ExternalInput").ap()
    woT = nc.dram_tensor("woT", [CH, D], F16, kind="ExternalInput").ap()
    cosT = nc.dram_tensor("cosT", [P, T], F16, kind="ExternalInput").ap()
    sinS = nc.dram_tensor("sinS", [P, T], F16, kind="ExternalInput").ap()
    ident = nc.dram_tensor("ident", [P, P], F16, kind="ExternalInput").ap()
    triB = nc.dram_tensor("triB", [P, P], F16, kind="ExternalInput").ap()
    onescol = nc.dram_tensor("onescol", [P, NKB * HPC], F16, kind="ExternalInput").ap()
    out = nc.dram_tensor("out", [T, D], F32, kind="ExternalOutput").ap()

    with tile.TileContext(nc) as tc:
        with tc.tile_pool(name="singles", bufs=1) as singles, \
             tc.tile_pool(name="rope16", bufs=3) as rope16, \
             tc.tile_pool(name="ptp", bufs=3) as ptp, \
             tc.tile_pool(name="obp", bufs=2) as obp, \
             tc.tile_pool(name="ssm", bufs=2) as ssm, \
             tc.tile_pool(name="bcp", bufs=3) as bcp, \
             tc.tile_pool(name="st_ps", bufs=3, space="PSUM") as stp, \
             tc.tile_pool(name="ot_ps", bufs=1, space="PSUM") as otp_pool:

            # ---- resident tiles ----
            xT_sb = singles.tile([P, DB, T], F16)
            wqT_sb = singles.tile([P, DB, CH], F16)
            wkT_sb = singles.tile([P, DB, CH], F16)
            wvT_sb = singles.tile([P, DB, CH], F16)
            woT_sb = singles.tile([P, CB, D], F16)
            cosT_sb = singles.tile([P, T], F16)
            sinS_sb = singles.tile([P, T], F16)
            ident_sb = singles.tile([P, P], F16)
            triB_sb = singles.tile([P, P], F16)
            QT_sb = singles.tile([P, CB, T], F16)
            KT_sb = singles.tile([P, CB, T], F16)
            attnT_sb = singles.tile([P, CB, T], F16)
            # V augmented to 128 columns: ones in col 0 (sums -> PSUM partition
            # 0 for the partition-0-only custom-DVE recip), V dims in cols
            # 64..127 (PSUM reads need 32-aligned partition offsets). Cols
            # 1..63 are never-read garbage.
            vaug = singles.tile([P, NKB, HPC, P], F16)

            xTr = xT.rearrange("(o p) t -> p o t", p=P)

            # ---- input DMA (issued up front, in consumption order) ----
            nc.sync.dma_start(wqT_sb[:], wqT.rearrange("(o p) c -> p o c", p=P))
            nc.sync.dma_start(xT_sb[:, :, 0:NQ], xTr[:, :, 0:NQ])
            nc.sync.dma_start(cosT_sb[:], cosT[:])
            nc.sync.dma_start(sinS_sb[:], sinS[:])
            nc.sync.dma_start(ident_sb[:], ident[:])
            nc.sync.dma_start(triB_sb[:], triB[:])
            nc.sync.dma_start(wkT_sb[:], wkT.rearrange("(o p) c -> p o c", p=P))
            nc.sync.dma_start(wvT_sb[:], wvT.rearrange("(o p) c -> p o c", p=P))
            # ones column FIRST so the PV sums row lands on PSUM partition 0
            # (custom-DVE recip and gpsimd broadcast require partition-0 input)
            nc.sync.dma_start(
                vaug[:, :, :, 0:1],
                onescol.rearrange("p (a b) -> p a b", a=NKB)[:, :, :, None])
            nc.sync.dma_start(xT_sb[:, :, NQ:2 * NQ], xTr[:, :, NQ:2 * NQ])
            nc.sync.dma_start(woT_sb[:], woT.rearrange("(o p) c -> p o c", p=P))
            nc.sync.dma_start(xT_sb[:, :, 2 * NQ:3 * NQ], xTr[:, :, 2 * NQ:3 * NQ])
            nc.sync.dma_start(xT_sb[:, :, 3 * NQ:4 * NQ], xTr[:, :, 3 * NQ:4 * NQ])

            # warm the exp table set while the DMAs land
            wtile = ssm.tile([1, 16], F32, tag="warm", bufs=1)
            nc.scalar.activation(wtile[:], sinS_sb[0:1, 0:16], AF.Exp, scale=1.0)

            # ---- projection of one q/k chunk (RoPE fused) ----
            def project(w_sb, dst_sb, cb, m, pname):
                ms = slice(m * NQ, (m + 1) * NQ)
                ps = stp.tile([P, 2, NQ], F32, tag="st",
                              name=f"prj_{pname}_{cb}_{m}")[:, 0, :]
                for o in range(DB):
                    nc.tensor.matmul(
                        ps[:],
                        w_sb[:, o, cb * P:(cb + 1) * P],
                        xT_sb[:, o, ms],
                        start=(o == 0), stop=(o == DB - 1))
                qraw = rope16.tile([P, NQ], F16, tag="qraw",
                                   name=f"qr_{pname}_{cb}_{m}")
                # DVE cast, not ACT copy: keeps RoPE off the exp-laden scalar
                # engine's queue (diag-block scores gate on this chain)
                nc.vector.tensor_copy(qraw[:], ps[:])
                # q' = q*cos + shift(q)*sinS  (shift = partner rows, sign in table)
                nc.vector.tensor_tensor(dst_sb[:, cb, ms], qraw[:],
                                        cosT_sb[:, ms], MUL)
                tmp = rope16.tile([P, NQ], F16, tag="tmp",
                                  name=f"tm_{pname}_{cb}_{m}")
                for g in range(4):
                    src = (g ^ 1) * 32
                    dst = g * 32
                    nc.vector.tensor_tensor(
                        tmp[dst:dst + 32, :],
                        qraw[src:src + 32, :],
                        sinS_sb[src:src + 32, ms], MUL)
                nc.gpsimd.tensor_tensor(dst_sb[:, cb, ms],
                                        dst_sb[:, cb, ms], tmp[:], ADD)

            def project_v(i):
                ps = stp.tile([P, 2, NQ], F32, tag="st", name=f"v_{i}")
                vps = ps[:, 0, :CH]
                for o in range(DB):
                    nc.tensor.matmul(
                        vps,
                        xT_sb[:, o, i * P:(i + 1) * P],
                        wvT_sb[:, o, :],
                        start=(o == 0), stop=(o == DB - 1))
                nc.vector.tensor_copy(
                    vaug[:, i, :, HD:2 * HD],
                    vps.rearrange("p (h d) -> p h d", h=HPC))

            def kb_list(qc):
                return list(range(min(NKB, (qc + 1) * (NQ // P)))) if causal \
                    else list(range(NKB))

            # ---- attention for one head-pair, one q-chunk ----
            def attention(hp, qc):
                kbs = kb_list(qc)
                q0 = qc * NQ
                otp = otp_pool.tile([P, 2, NQ], F32, tag="ot",
                                    name=f"ot_{hp}_{qc}")

                def finish(kb, stp2, qsl):
                    pt = ptp.tile([P, 2, NQ], F16, tag="pt",
                                  name=f"pt_{hp}_{qc}_{kb}")
                    # strided exp skips the causally-dead columns of both halves
                    nc.scalar.activation(pt[:, :, qsl:], stp2[:, :, qsl:],
                                         AF.Exp, scale=float(HD) ** -0.5)
                    for half in range(2):
                        h = hp * 2 + half
                        nc.tensor.matmul(
                            otp[:, half, qsl:],
                            vaug[:, kb, h, :],
                            pt[:, half, qsl:],
                            start=(kb == kbs[0]), stop=(kb == kbs[-1]))

                pend = []
                for kb in kbs:
                    qsl = max(0, kb * P - q0) if causal else 0
                    diag = causal and kb * P >= q0
                    stp2 = stp.tile([P, 2, NQ], F32, tag="st",
                                    name=f"st_{hp}_{qc}_{kb}")
                    for half in range(2):
                        hb = half * HD
                        nc.tensor.matmul(
                            stp2[:, half, qsl:],
                            KT_sb[hb:hb + HD, hp, kb * P:(kb + 1) * P],
                            QT_sb[hb:hb + HD, hp, q0 + qsl:q0 + NQ],
                            start=True, stop=not diag)
                    if diag:
                        # causal mask: add -30000 strictly below the diagonal
                        # so exp underflows those to zero
                        for half in range(2):
                            nc.tensor.matmul(
                                stp2[:, half, qsl:qsl + P],
                                ident_sb[:],
                                triB_sb[:],
                                start=False, stop=True)
                    pend.append((kb, stp2, qsl))
                    if len(pend) > 2:
                        finish(*pend.pop(0))
                while pend:
                    finish(*pend.pop(0))
                return otp

            def normalize(hp, qc, otp):
                for half in range(2):
                    rcd = ssm.tile([1, NQ], F32, tag="rcd",
                                   name=f"rcd_{hp}_{qc}_{half}")
                    nc.vector.reciprocal_approx_fast(rcd[:],
                                                     otp[0:1, half, :])
                    bc = bcp.tile([HD, NQ], F32, tag="bc",
                                  name=f"bc_{hp}_{qc}_{half}")
                    nc.gpsimd.partition_broadcast(bc[:], rcd[:])
                    nc.vector.tensor_tensor(
                        attnT_sb[half * HD:(half + 1) * HD, hp,
                                 qc * NQ:(qc + 1) * NQ],
                        otp[HD:2 * HD, half, :], bc[:], MUL)

            def wo_chunk(qc):
                for i in range(4 * qc, 4 * qc + 4):
                    ps = stp.tile([P, 2, NQ], F32, tag="st", name=f"o_{i}")
                    for cb in range(CB):
                        for j in range(2):
                            nc.tensor.matmul(
                                ps[:, j, :],
                                attnT_sb[:, cb, i * P:(i + 1) * P],
                                woT_sb[:, cb, j * NQ:(j + 1) * NQ],
                                start=(cb == 0), stop=(cb == CB - 1))
                    ob = obp.tile([P, 2 * NQ], F32, tag="ob", name=f"ob_{i}")
                    nc.vector.tensor_copy(
                        ob.rearrange("p (a b) -> p a b", a=2), ps[:])
                    nc.sync.dma_start(out[i * P:(i + 1) * P, :], ob[:])

            # ---- fused pipeline ----
            for m in range(NT):
                project(wqT_sb, QT_sb, 0, m, "q")
                project(wqT_sb, QT_sb, 1, m, "q")
                project(wkT_sb, KT_sb, 0, m, "k")
                project(wkT_sb, KT_sb, 1, m, "k")
                for i in range(4 * m, 4 * m + 4):
                    project_v(i)
                if m > 0:
                    wo_chunk(m - 1)
                ot0 = attention(0, m)
                normalize(0, m, ot0)
                ot1 = attention(1, m)
                normalize(1, m, ot1)
            wo_chunk(NT - 1)

    nc.compile()
    return nc


def _get_program(causal: bool):
    key = ("causal" if causal else "full")
    if key not in _prog_cache:
        _prog_cache[key] = _build_program(causal)
    return _prog_cache[key]


def _mask_kind(mask):
    m = np.asarray(mask)
    if m.ndim == 4:
        m = m[0, 0]
    if (m != 0).all():
        return False  # full attention
    trilm = np.tril(np.ones((m.shape[0], m.shape[1]), dtype=m.dtype))
    if np.array_equal(m, trilm):
        return True
    raise NotImplementedError("mask is neither all-ones nor causal tril")


def _make_in_maps(x, cos, sin, wq, wk, wv, wo):
    x = np.asarray(x, dtype=np.float32)
    cos = np.asarray(cos, dtype=np.float32)
    sin = np.asarray(sin, dtype=np.float32)
    wq = np.asarray(wq, dtype=np.float32)
    wk = np.asarray(wk, dtype=np.float32)
    wv = np.asarray(wv, dtype=np.float32)
    wo = np.asarray(wo, dtype=np.float32)

    # RoPE tables in transposed head-pair layout [128ch, T].
    # cos2T[c, t] = cos[t, c % 64]; sinsgn flips sign on the low half of each head;
    # sinS is additionally row-swapped (c ^ 32) so the shifted multiply can read
    # both inputs from the same base partition.
    ci = np.arange(P) % HD
    cos2T = np.ascontiguousarray(cos[:T, ci].T.astype(np.float16))  # [128, T]
    sgn = np.where((np.arange(P) % HD) < (HD // 2), -1.0, 1.0).astype(np.float32)
    sinsgn = sin[:T, ci].T * sgn[:, None]                      # [128, T]
    sinS = np.ascontiguousarray(
        sinsgn[np.arange(P) ^ 32, :].astype(np.float16))       # row-swapped
    identm = np.eye(P, dtype=np.float16)
    triBm = np.ascontiguousarray(
        (np.tril(np.ones((P, P), np.float32), -1) * -30000.0).astype(np.float16))
    ones = np.ones((P, (T // P) * HPC), dtype=np.float16)

    in_maps = []
    for core in range(NCORES):
        b = core // GROUPS
        g = core % GROUPS
        c0 = g * CH
        in_maps.append({
            "xT": np.ascontiguousarray(x[b].T.astype(np.float16)),          # [D, T]
            "wqT": np.ascontiguousarray(wq[c0:c0 + CH, :].T.astype(np.float16)),
            "wkT": np.ascontiguousarray(wk[c0:c0 + CH, :].T.astype(np.float16)),
            "wvT": np.ascontiguousarray(wv[c0:c0 + CH, :].T.astype(np.float16)),
            "woT": np.ascontiguousarray(wo[:, c0:c0 + CH].T.astype(np.float16)),
            "cosT": cos2T,
            "sinS": sinS,
            "ident": identm,
            "triB": triBm,
            "onescol": ones,
        })
    return in_maps


def _run(inputs, trace=False):
    from concourse import bass_utils
    causal = _mask_kind(inputs["mask"])
    nc = _get_program(causal)
    in_maps = _make_in_maps(
        inputs["x"], inputs["cos"], inputs["sin"],
        inputs["wq"], inputs["wk"], inputs["wv"], inputs["wo"])
    if trace:
        _install_ntff_shim()
    res = bass_utils.run_bass_kernel_spmd(
        nc, in_maps, core_ids=list(range(NCORES)), trace=trace)
    outs = [r["out"] for r in res.results]
    full = np.empty((B, T, D), dtype=np.float32)
    for b in range(B):
        full[b] = outs[b * GROUPS]
        for g in range(1, GROUPS):
            full[b] += outs[b * GROUPS + g]
    return full, res


def kernel(**inputs):
    full, _ = _run(inputs, trace=False)
    return full


def kernel_profiled(**inputs):
    """Like kernel() but with NTFF tracing; returns (out, BassKernelResults)."""
    return _run(inputs, trace=True)


# revision 16
# speedup vs baseline: 1.2147x; 1.2147x over previous
"""Trainium2 Bass kernel for nn_Attention_47682726920277.

Causal multi-head attention with RoPE:
  q/k/v = x @ w{q,k,v}.T ; RoPE(q, k) ; att = softmax(mask(q k^T / 8)) ; out = (att v) @ wo.T
Shapes: x [2, 2048, 1024], 16 heads of dim 64, fp32.

Sharding (8 cores): data-parallel over batch (2) x tensor-parallel over heads (4 per
core). Each core computes its 4 heads' attention and a partial out via its wo row
block; the final all-reduce is the host-side sum of the 4 partials per batch.

Per-core pipeline (single fused program):
  for m in 0..3 (query chunks of 512):
    project+RoPE q,k chunk m (both head-pairs); project v blocks 4m..4m+3
    attention(hp0, qc=m); normalize(hp0); wo chunk m-1; attention(hp1, qc=m);
    normalize(hp1)
  wo chunk 3
Scores come out transposed (S^T [k, q]) so the softmax denominator is the 65th
row of the PV accumulator (V augmented with a ones column). exp runs on the
scalar engine straight out of PSUM with the 1/8 scale fused and a strided access
pattern that skips causally-masked columns. Normalization is
reciprocal_approx_fast (DVE) + gpsimd partition-broadcast + one DVE multiply
reading the PV accumulator directly from PSUM. Score half-pairs row-pack onto
the PE array (tile_position auto via base partitions 0/64).
"""
import sys
import types
import numpy as np

B = 2
T = 2048
D = 1024
H = 16
HD = 64
NCORES = 8
GROUPS = NCORES // B          # head-groups per batch
HPC = H // GROUPS             # heads per core = 4
CH = HPC * HD                 # channels per core = 256
NQ = 512                      # PSUM bank width (fp32)
P = 128

_prog_cache = {}


def _install_ntff_shim():
    """The agent image's antenv lacks axon_hooks; inject it so trace=True works."""
    try:
        import antenv.axon_hooks  # noqa: F401
        return
    except ImportError:
        pass
    try:
        import trn_agent_boot.trn_boot as tb
        hook = tb._ntff_profile_via_ctypes('/opt/axon/libaxon_pjrt.so')
        if hook is None:
            return
        mod = types.ModuleType('antenv.axon_hooks')
        mod.get_axon_ntff_profile_hook = lambda: hook
        mod.set_axon_ntff_profile_hook = lambda h: None
        sys.modules['antenv.axon_hooks'] = mod
        import antenv
        antenv.axon_hooks = mod
    except Exception:
        pass


def _build_program(causal: bool):
    import concourse.bass as bass  # noqa: F401
    from concourse import bacc
    import concourse.tile as tile
    from concourse import mybir

    F32 = mybir.dt.float32
    F16 = mybir.dt.float16
    AF = mybir.ActivationFunctionType
    MUL = mybir.AluOpType.mult
    ADD = mybir.AluOpType.add

    NT = T // NQ          # proj/attention q-chunks (4)
    NKB = T // P          # k-blocks (16)
    DB = D // P           # d-blocks (8)
    CB = CH // P          # channel blocks = head-pair blocks (2)

    nc = bacc.Bacc("TRN2", target_bir_lowering=False, debug=False)

    xT = nc.dram_tensor("xT", [D, T], F16, kind="ExternalInput").ap()
    wqT = nc.dram_tensor("wqT", [D, CH], F16, kind="ExternalInput").ap()
    wkT = nc.dram_tensor("wkT", [D, CH], F16, kind="ExternalInput").ap()
    wvT = nc.dram_tensor("wvT", [D, CH], F16, kind="ExternalInput").ap()
    woT = nc.dram_tensor("woT", [CH, D], F16, kind="ExternalInput").ap()
    cosT = nc.dram_tensor("cosT", [P, T], F16, kind="ExternalInput").ap()
    sinS = nc.dram_tensor("sinS", [P, T], F16, kind="ExternalInput").ap()
    ident = nc.dram_tensor("ident", [P, P], F16, kind="ExternalInput").ap()
    triB = nc.dram_tensor("triB", [P, P], F16, kind="ExternalInput").ap()
    onescol = nc.dram_tensor("onescol", [P, NKB * HPC], F16, kind="ExternalInput").ap()
    out = nc.dram_tensor("out", [T, D], F32, kind="ExternalOutput").ap()

    with tile.TileContext(nc) as tc:
        with tc.tile_pool(name="singles", bufs=1) as singles, \
             tc.tile_pool(name="rope16", bufs=3) as rope16, \
             tc.tile_pool(name="ptp", bufs=3) as ptp, \
             tc.tile_pool(name="obp", bufs=2) as obp, \
             tc.tile_pool(name="ssm", bufs=2) as ssm, \
             tc.tile_pool(name="bcp", bufs=3) as bcp, \
             tc.tile_pool(name="st_ps", bufs=3, space="PSUM") as stp, \
             tc.tile_pool(name="ot_ps", bufs=1, space="PSUM") as otp_pool:

            # ---- resident tiles ----
            xT_sb = singles.tile([P, DB, T], F16)
            wqT_sb = singles.tile([P, DB, CH], F16)
            wkT_sb = singles.tile([P, DB, CH], F16)
            wvT_sb = singles.tile([P, DB, CH], F16)
            woT_sb = singles.tile([P, CB, D], F16)
            cosT_sb = singles.tile([P, T], F16)
            sinS_sb = singles.tile([P, T], F16)
            ident_sb = singles.tile([P, P], F16)
            triB_sb = singles.tile([P, P], F16)
            QT_sb = singles.tile([P, CB, T], F16)
            KT_sb = singles.tile([P, CB, T], F16)
            attnT_sb = singles.tile([P, CB, T], F16)
            # V augmented to 128 columns: ones in col 0 (sums -> PSUM partition
            # 0 for the partition-0-only custom-DVE recip), V dims in cols
            # 64..127 (PSUM reads need 32-aligned partition offsets). Cols
            # 1..63 are never-read garbage.
            vaug = singles.tile([P, NKB, HPC, P], F16)

            xTr = xT.rearrange("(o p) t -> p o t", p=P)

            # ---- input DMA (issued up front, in consumption order) ----
            nc.sync.dma_start(wqT_sb[:], wqT.rearrange("(o p) c -> p o c", p=P))
            nc.sync.dma_start(xT_sb[:, :, 0:NQ], xTr[:, :, 0:NQ])
            nc.sync.dma_start(cosT_sb[:], cosT[:])
            nc.sync.dma_start(sinS_sb[:], sinS[:])
            nc.sync.dma_start(ident_sb[:], ident[:])
            nc.sync.dma_start(triB_sb[:], triB[:])
            nc.sync.dma_start(wkT_sb[:], wkT.rearrange("(o p) c -> p o c", p=P))
            nc.sync.dma_start(wvT_sb[:], wvT.rearrange("(o p) c -> p o c", p=P))
            # ones column FIRST so the PV sums row lands on PSUM partition 0
            # (custom-DVE recip and gpsimd broadcast require partition-0 input)
            nc.sync.dma_start(
                vaug[:, :, :, 0:1],
                onescol.rearrange("p (a b) -> p a b", a=NKB)[:, :, :, None])
            nc.sync.dma_start(xT_sb[:, :, NQ:2 * NQ], xTr[:, :, NQ:2 * NQ])
            nc.sync.dma_start(woT_sb[:], woT.rearrange("(o p) c -> p o c", p=P))
            nc.sync.dma_start(xT_sb[:, :, 2 * NQ:3 * NQ], xTr[:, :, 2 * NQ:3 * NQ])
            nc.sync.dma_start(xT_sb[:, :, 3 * NQ:4 * NQ], xTr[:, :, 3 * NQ:4 * NQ])

            # warm the exp table set while the DMAs land
            wtile = ssm.tile([1, 16], F32, tag="warm", bufs=1)
            nc.scalar.activation(wtile[:], sinS_sb[0:1, 0:16], AF.Exp, scale=1.0)

            # ---- projection of one q/k chunk (RoPE fused) ----
            def project(w_sb, dst_sb, cb, m, pname):
                ms = slice(m * NQ, (m + 1) * NQ)
                ps = stp.tile([P, 2, NQ], F32, tag="st",
                              name=f"prj_{pname}_{cb}_{m}")[:, 0, :]
                for o in range(DB):
                    nc.tensor.matmul(
                        ps[:],
                        w_sb[:, o, cb * P:(cb + 1) * P],
                        xT_sb[:, o, ms],
                        start=(o == 0), stop=(o == DB - 1))
                qraw = rope16.tile([P, NQ], F16, tag="qraw",
                                   name=f"qr_{pname}_{cb}_{m}")
                # DVE cast, not ACT copy: keeps RoPE off the exp-laden scalar
                # engine's queue (diag-block scores gate on this chain)
                nc.vector.tensor_copy(qraw[:], ps[:])
                # q' = q*cos + shift(q)*sinS  (shift = partner rows, sign in table)
                nc.vector.tensor_tensor(dst_sb[:, cb, ms], qraw[:],
                                        cosT_sb[:, ms], MUL)
                tmp = rope16.tile([P, NQ], F16, tag="tmp",
                                  name=f"tm_{pname}_{cb}_{m}")
                for g in range(4):
                    src = (g ^ 1) * 32
                    dst = g * 32
                    nc.vector.tensor_tensor(
                        tmp[dst:dst + 32, :],
                        qraw[src:src + 32, :],
                        sinS_sb[src:src + 32, ms], MUL)
                # DVE, not gpsimd: the gpsimd queue carries normalize's
                # broadcasts, which would gate this add on the previous
                # attention section's exp backlog
                nc.vector.tensor_tensor(dst_sb[:, cb, ms],
                                        dst_sb[:, cb, ms], tmp[:], ADD)

            def project_v(i):
                ps = stp.tile([P, 2, NQ], F32, tag="st", name=f"v_{i}")
                vps = ps[:, 0, :CH]
                for o in range(DB):
                    nc.tensor.matmul(
                        vps,
                        xT_sb[:, o, i * P:(i + 1) * P],
                        wvT_sb[:, o, :],
                        start=(o == 0), stop=(o == DB - 1))
                nc.vector.tensor_copy(
                    vaug[:, i, :, HD:2 * HD],
                    vps.rearrange("p (h d) -> p h d", h=HPC))

            def kb_list(qc):
                return list(range(min(NKB, (qc + 1) * (NQ // P)))) if causal \
                    else list(range(NKB))

            # ---- attention for one head-pair, one q-chunk ----
            def attention(hp, qc):
                kbs = kb_list(qc)
                q0 = qc * NQ
                otp = otp_pool.tile([P, 2, NQ], F32, tag="ot",
                                    name=f"ot_{hp}_{qc}")

                def finish(kb, stp2, qsl):
                    pt = ptp.tile([P, 2, NQ], F16, tag="pt",
                                  name=f"pt_{hp}_{qc}_{kb}")
                    # strided exp skips the causally-dead columns of both halves
                    nc.scalar.activation(pt[:, :, qsl:], stp2[:, :, qsl:],
                                         AF.Exp, scale=float(HD) ** -0.5)
                    for half in range(2):
                        h = hp * 2 + half
                        nc.tensor.matmul(
                            otp[:, half, qsl:],
                            vaug[:, kb, h, :],
                            pt[:, half, qsl:],
                            start=(kb == kbs[0]), stop=(kb == kbs[-1]))

                pend = []
                for kb in kbs:
                    qsl = max(0, kb * P - q0) if causal else 0
                    diag = causal and kb * P >= q0
                    stp2 = stp.tile([P, 2, NQ], F32, tag="st",
                                    name=f"st_{hp}_{qc}_{kb}")
                    for half in range(2):
                        hb = half * HD
                        nc.tensor.matmul(
                            stp2[:, half, qsl:],
                            KT_sb[hb:hb + HD, hp, kb * P:(kb + 1) * P],
                            QT_sb[hb:hb + HD, hp, q0 + qsl:q0 + NQ],
                            start=True, stop=not diag)
                    if diag:
                        # causal mask: add -30000 strictly below the diagonal
                        # so exp underflows those to zero
                        for half in range(2):
                            nc.tensor.matmul(
                                stp2[:, half, qsl:qsl + P],
                                ident_sb[:],
                                triB_sb[:],
                                start=False, stop=True)
                    pend.append((kb, stp2, qsl))
                    if len(pend) > 2:
                        finish(*pend.pop(0))
                while pend:
                    finish(*pend.pop(0))
                return otp

            def normalize(hp, qc, otp):
                for half in range(2):
                    rcd = ssm.tile([1, NQ], F32, tag="rcd",
                                   name=f"rcd_{hp}_{qc}_{half}")
                    nc.vector.reciprocal_approx_fast(rcd[:],
                                                     otp[0:1, half, :])
                    bc = bcp.tile([HD, NQ], F32, tag="bc",
                                  name=f"bc_{hp}_{qc}_{half}")
                    nc.gpsimd.partition_broadcast(bc[:], rcd[:])
                    nc.vector.tensor_tensor(
                        attnT_sb[half * HD:(half + 1) * HD, hp,
                                 qc * NQ:(qc + 1) * NQ],
                        otp[HD:2 * HD, half, :], bc[:], MUL)

            def wo_chunk(qc):
                for i in range(4 * qc, 4 * qc + 4):
                    ps = stp.tile([P, 2, NQ], F32, tag="st", name=f"o_{i}")
                    for cb in range(CB):
                        for j in range(2):
                            nc.tensor.matmul(
                                ps[:, j, :],
                                attnT_sb[:, cb, i * P:(i + 1) * P],
                                woT_sb[:, cb, j * NQ:(j + 1) * NQ],
                                start=(cb == 0), stop=(cb == CB - 1))
                    ob = obp.tile([P, 2 * NQ], F32, tag="ob", name=f"ob_{i}")
                    nc.vector.tensor_copy(
                        ob.rearrange("p (a b) -> p a b", a=2), ps[:])
                    nc.sync.dma_start(out[i * P:(i + 1) * P, :], ob[:])

            # ---- fused pipeline ----
            for m in range(NT):
                project(wqT_sb, QT_sb, 0, m, "q")
                project(wqT_sb, QT_sb, 1, m, "q")
                project(wkT_sb, KT_sb, 0, m, "k")
                project(wkT_sb, KT_sb, 1, m, "k")
                for i in range(4 * m, 4 * m + 4):
                    project_v(i)
                ot0 = attention(0, m)
                normalize(0, m, ot0)
                if m > 0:
                    wo_chunk(m - 1)
                ot1 = attention(1, m)
                normalize(1, m, ot1)
            wo_chunk(NT - 1)

    nc.compile()
    return nc


def _get_program(causal: bool):
    key = ("causal" if causal else "full")
    if key not in _prog_cache:
        _prog_cache[key] = _build_program(causal)
    return _prog_cache[key]


def _mask_kind(mask):
    m = np.asarray(mask)
    if m.ndim == 4:
        m = m[0, 0]
    if (m != 0).all():
        return False  # full attention
    trilm = np.tril(np.ones((m.shape[0], m.shape[1]), dtype=m.dtype))
    if np.array_equal(m, trilm):
        return True
    raise NotImplementedError("mask is neither all-ones nor causal tril")


def _make_in_maps(x, cos, sin, wq, wk, wv, wo):
    x = np.asarray(x, dtype=np.float32)
    cos = np.asarray(cos, dtype=np.float32)
    sin = np.asarray(sin, dtype=np.float32)
    wq = np.asarray(wq, dtype=np.float32)
    wk = np.asarray(wk, dtype=np.float32)
    wv = np.asarray(wv, dtype=np.float32)
    wo = np.asarray(wo, dtype=np.float32)

    # RoPE tables in transposed head-pair layout [128ch, T].
    # cos2T[c, t] = cos[t, c % 64]; sinsgn flips sign on the low half of each head;
    # sinS is additionally row-swapped (c ^ 32) so the shifted multiply can read
    # both inputs from the same base partition.
    ci = np.arange(P) % HD
    cos2T = np.ascontiguousarray(cos[:T, ci].T.astype(np.float16))  # [128, T]
    sgn = np.where((np.arange(P) % HD) < (HD // 2), -1.0, 1.0).astype(np.float32)
    sinsgn = sin[:T, ci].T * sgn[:, None]                      # [128, T]
    sinS = np.ascontiguousarray(
        sinsgn[np.arange(P) ^ 32, :].astype(np.float16))       # row-swapped
    identm = np.eye(P, dtype=np.float16)
    triBm = np.ascontiguousarray(
        (np.tril(np.ones((P, P), np.float32), -1) * -30000.0).astype(np.float16))
    ones = np.ones((P, (T // P) * HPC), dtype=np.float16)

    in_maps = []
    for core in range(NCORES):
        b = core // GROUPS
        g = core % GROUPS
        c0 = g * CH
        in_maps.append({
            "xT": np.ascontiguousarray(x[b].T.astype(np.float16)),          # [D, T]
            "wqT": np.ascontiguousarray(wq[c0:c0 + CH, :].T.astype(np.float16)),
            "wkT": np.ascontiguousarray(wk[c0:c0 + CH, :].T.astype(np.float16)),
            "wvT": np.ascontiguousarray(wv[c0:c0 + CH, :].T.astype(np.float16)),
            "woT": np.ascontiguousarray(wo[:, c0:c0 + CH].T.astype(np.float16)),
            "cosT": cos2T,
            "sinS": sinS,
            "ident": identm,
            "triB": triBm,
            "onescol": ones,
        })
    return in_maps


def _run(inputs, trace=False):
    from concourse import bass_utils
    causal = _mask_kind(inputs["mask"])
    nc = _get_program(causal)
    in_maps = _make_in_maps(
        inputs["x"], inputs["cos"], inputs["sin"],
        inputs["wq"], inputs["wk"], inputs["wv"], inputs["wo"])
    if trace:
        _install_ntff_shim()
    res = bass_utils.run_bass_kernel_spmd(
        nc, in_maps, core_ids=list(range(NCORES)), trace=trace)
    outs = [r["out"] for r in res.results]
    full = np.empty((B, T, D), dtype=np.float32)
    for b in range(B):
        full[b] = outs[b * GROUPS]
        for g in range(1, GROUPS):
            full[b] += outs[b * GROUPS + g]
    return full, res


def kernel(**inputs):
    full, _ = _run(inputs, trace=False)
    return full


def kernel_profiled(**inputs):
    """Like kernel() but with NTFF tracing; returns (out, BassKernelResults)."""
    return _run(inputs, trace=True)
